# revision 43
# baseline (speedup 1.0000x reference)
"""Trainium2 Bass kernel for nn_Attention (dense transformer cross-attention).

Strategy: data-parallel over batch (B=8) -> one batch element per NeuronCore.
Per core, everything is computed with zero on-chip transposes by choosing
layouts up front (host pre-transposes activations/weights, which is free):

  K^T_h [dh=128, M]   = Wk-chunk^T . memory^T   (per head h, bias via ACT)
  Q^T_h [dh=128, Q]   = (scaled Wq)-chunk^T . query^T
  V     [M, D]        = memory . Wv^T + bv      (natural layout)
  S^T   [M, Q]        = K^T_h^T-free-slices . Q^T_h
  expS  = ACT Exp with per-partition mask bias (-1e30 -> exact 0), bf16
  Z     = two half running DVE adds over m-chunks; cross-partition totals
          via accumulating one-hot matmuls -> [8,128] PSUM (q-major)
  rb    = 1/Z: tiny DVE reciprocal, bounced through DRAM bf16 with a
          contiguous write + partition-broadcast read -> [128, Q]
  ctx^T_h [dh, Q]     = V-chunks . expS  (PSUM accum over m-chunks),
                        normalization fused into the PSUM->SBUF drain
  out   [Q, D]        = ctx^T (as lhsT, heads = contraction chunks) . Wf^T
                        + bf (fused into final drain)

Pipeline shape: K proj | Q proj | V proj | per-head attention [scores h ->
exp h with head h-1's PV matmuls and deferred normalization chain riding
the same stretch; p0 chunks spread across heads] | tail PV | final.

m-chunks that are fully masked in EVERY batch element (host-inspected) are
skipped everywhere: K-proj columns, V rows, scores, exp, PV, and p0 rows
(output buffers are pre-zeroed by the runtime). Arbitrary masks still work
via the per-chunk exp bias; the skip set only prunes provably-dead work.

Softmax max-subtraction is skipped: scores are O(1) by construction
(0.02-scale weights), exp is computed in f32 on ACT, so this is exact.

Compute dtype bf16 (f32 PSUM accumulation); inputs converted on host so
device DMA moves half the bytes. p0 is emitted bf16 (probs fit easily).
"""

import math

import numpy as np
import ml_dtypes

B = 8
Q = 1024
M = 1024
D = 1024
H = 8
DH = 128
KC = 8  # 128-row contraction chunks per 1024
NT = 2  # 512-wide free tiles per 1024
FT = 512

_BF16 = ml_dtypes.bfloat16
_CACHE = {}


def _build_program(reps=1, parts='paf', norm=True, skip=()):
    import concourse.bass as bass
    import concourse.mybir as mybir
    from concourse.tile import TileContext

    import bass_rust

    f32 = mybir.dt.float32
    bf16 = mybir.dt.bfloat16
    Identity = mybir.ActivationFunctionType.Identity
    Exp = mybir.ActivationFunctionType.Exp

    skip = frozenset(skip)
    keep = [c for c in range(KC) if c not in skip]
    nk = len(keep)
    assert nk >= 2, "degenerate mask"
    halfA = (nk + 1) // 2
    # contiguous kept-m runs and <=FT tiles over them (K-proj free dim)
    runs = []
    for c in keep:
        if runs and runs[-1][1] == c * DH:
            runs[-1][1] = (c + 1) * DH
        else:
            runs.append([c * DH, (c + 1) * DH])
    m_tiles = []
    slot = 0
    for lo, hi in runs:
        t0 = lo
        while t0 < hi:
            n = min(FT, hi - t0)
            m_tiles.append((t0, slot, n))
            t0 += n
            slot += n
    MK = nk * DH  # compacted (kept) m extent

    def split_sync_waits(nc):
        """The walrus in this container accepts only ONE sync-wait per
        instruction; Tile freely attaches several. Move excess waits onto
        same-engine NOPs spliced immediately before the instruction."""
        for fn in nc.m.functions:
            for bb in fn.blocks:
                out = []
                for inst in bb.instructions:
                    si = inst.sync_info
                    if si is not None and si.on_wait is not None and len(si.on_wait) > 1:
                        waits = list(si.on_wait)
                        si.on_wait = waits[-1:]
                        for j, w in enumerate(waits[:-1]):
                            nop = bass_rust.InstNoOp(
                                name=f"{inst.name}_sw{j}", ins=[], outs=[])
                            nop.engine = inst.engine
                            nop.sync_info = mybir.SyncInfo(on_wait=[w], on_update=[])
                            out.append(nop)
                    out.append(inst)
                bb.instructions = out

    nc = bass.Bass()

    memT = nc.declare_dram_parameter("memT", [D, M], bf16, isOutput=False)
    qT = nc.declare_dram_parameter("qT", [D, Q], bf16, isOutput=False)
    wkT = nc.declare_dram_parameter("wkT", [D, D], bf16, isOutput=False)
    wvT = nc.declare_dram_parameter("wvT", [D, D], bf16, isOutput=False)
    wqT = nc.declare_dram_parameter("wqT", [D, D], bf16, isOutput=False)
    wfT = nc.declare_dram_parameter("wfT", [D, D], bf16, isOutput=False)
    bk_pp = nc.declare_dram_parameter("bk_pp", [128, H], f32, isOutput=False)
    bq_pp = nc.declare_dram_parameter("bq_pp", [128, H], f32, isOutput=False)
    mb_pp = nc.declare_dram_parameter("mb_pp", [128, KC], f32, isOutput=False)
    bv_bc = nc.declare_dram_parameter("bv_bc", [128, D], bf16, isOutput=False)
    bf_bc = nc.declare_dram_parameter("bf_bc", [128, D], bf16, isOutput=False)

    wm = nc.declare_dram_parameter("wm", [Q, D], f32, isOutput=True)
    p0t = nc.declare_dram_parameter("p0t", [M, Q], bf16, isOutput=True)

    def chunked(dram_ap):
        # [1024, N] DRAM -> [p=128, c=8, N] access pattern
        return dram_ap.rearrange("(c p) n -> p c n", p=128)

    with TileContext(nc) as tc:
      for rep in range(reps):
        with tc.tile_pool(name=f"const{rep}", bufs=1) as const, \
             tc.tile_pool(name=f"persist{rep}", bufs=1) as persist:
            wf_sb = (const.tile([128, KC, D], bf16, name="wf_sb")
                     if skip else None)
            bkt = const.tile([128, H], f32)
            bqt = const.tile([128, H], f32)
            mbt = const.tile([128, KC], f32)
            onehot = const.tile([128, KC, KC], bf16)

            nc.scalar.dma_start(out=bkt[:], in_=bk_pp[:, :])
            nc.scalar.dma_start(out=bqt[:], in_=bq_pp[:, :])
            nc.scalar.dma_start(out=mbt[:], in_=mb_pp[:, :])
            nc.vector.memset(onehot[:], 0.0)
            for c in range(KC):
                nc.vector.memset(onehot[:, c, c:c + 1], 1.0)

            k_sb = persist.tile([128, H, MK], bf16)
            q_sb = persist.tile([128, H, Q], bf16)
            v_sb = persist.tile([128, nk, D], bf16)
            ctx_sb = persist.tile([128, H, Q], bf16)

            if True:
                rb_holder = {}
                zt_holder = {}
                pools = {}
                exp_holder = {}
                runAB = {}

                def emit_pv(ph, pexp, cp_tiles, idx):
                    # one PV matmul of the software-pipelined previous head;
                    # cc-major so both t-tiles of a chunk share the stationary
                    ci, t_ = divmod(idx, NT)
                    ts_ = slice(t_ * FT, (t_ + 1) * FT)
                    phs = slice(ph * DH, (ph + 1) * DH)
                    if idx == 0:
                        for t2 in range(NT):
                            cp_tiles[t2] = pools['cp'].tile(
                                [128, FT], f32, tag="cp",
                                name=f"cp_h{ph}_t{t2}")
                    nc.tensor.matmul(
                        cp_tiles[t_][:], v_sb[:, ci, phs], pexp[:, ci, ts_],
                        start=(ci == 0), stop=(ci == nk - 1))
                    if ci == nk - 1:
                        if norm:
                            nc.vector.tensor_mul(
                                ctx_sb[:, ph, ts_], cp_tiles[t_][:],
                                rb_holder[ph][:, ts_])
                        else:
                            nc.vector.tensor_copy(ctx_sb[:, ph, ts_], cp_tiles[t_][:])

                def emit_p0_chunk(pexp, prb, ci):
                    # head-0 attention-weights output (tail; runs during the
                    # final projection where DVE has slack). all-bf16 -> 2x.
                    # fully-masked chunks: p0 rows are exact zeros, and the
                    # runtime pre-zeroes output buffers -> nothing to write
                    c = keep[ci]
                    p0_sb = pools['attn3'].tile([128, Q], bf16, tag="p0",
                                                bufs=1)
                    with nc.allow_low_precision(reason="p0 probs fit bf16"):
                        nc.vector.tensor_mul(p0_sb[:], pexp[:, ci, :], prb[:])
                    nc.sync.dma_start(
                        out=p0t[c * DH:(c + 1) * DH, :], in_=p0_sb[:])

                def emit_zmms(h, sum_ap, first, last):
                    # cross-partition totals accumulate into a [8,128] PSUM
                    # tile via one-hot stationaries (q-major layout, so the
                    # later DRAM bounce is a contiguous write)
                    ztT = zt_holder.get(h)
                    if ztT is None:
                        ztT = pools['zt'].tile([KC, DH], f32, tag="ztT",
                                               bufs=1, name=f"ztT_h{h}")
                        zt_holder[h] = ztT
                    for qc in range(KC):
                        qs = slice(qc * DH, (qc + 1) * DH)
                        nc.tensor.matmul(
                            ztT[:, :], onehot[:, qc, :], sum_ap[:, qs],
                            start=(first and qc == 0),
                            stop=(last and qc == KC - 1))

                def finish_norm(h):
                    ztT = zt_holder.pop(h)
                    rzT = pools['attn'].tile([KC, DH], bf16, tag="rzT",
                                             name=f"rzT_h{h}")
                    with nc.allow_low_precision(reason="1/Z fits bf16"):
                        nc.vector.reciprocal(rzT[:], ztT[:])
                    zd = pools['dramp'].tile([KC, DH], bf16, tag="zd",
                                             name=f"zd_h{h}")
                    nc.scalar.dma_start(out=zd[:, :], in_=rzT[:])
                    # rb_h0 is read by the tail p0 ops -> own non-cycling tag
                    rb = pools['attn'].tile([128, Q], bf16,
                                            tag="rb0" if h == 0 else "rb",
                                            bufs=1 if h == 0 else 2,
                                            name=f"rb_h{h}")
                    nc.sync.dma_start(
                        out=rb[:],
                        in_=zd[:, :].rearrange("c j -> (c j)")[None, :]
                            .broadcast_to([128, Q]))
                    rb_holder[h] = rb

                def head_chunk(h, ci, prev_exp, cp_tiles, pending):
                    """scores+exp for (h, keep[ci]) plus the riding work:
                    previous head's PV pair and deferred norm chain."""
                    c = keep[ci]
                    cs = slice(ci * DH, (ci + 1) * DH)
                    exp_sb = exp_holder[h]
                    if prev_exp is not None:
                        emit_pv(h - 1, prev_exp, cp_tiles, 2 * ci)
                    st = pools['st'].tile([128, Q], f32, tag="st",
                                          name=f"st_h{h}_c{ci}")
                    for t in range(NT):
                        ts_ = slice(t * FT, (t + 1) * FT)
                        nc.tensor.matmul(
                            st[:, ts_], k_sb[:, h, cs], q_sb[:, h, ts_],
                            start=True, stop=True)
                    nc.scalar.activation(
                        exp_sb[:, ci, :], st[:], Exp, bias=mbt[:, c:c + 1])
                    if ci == 1 and pending is not None:
                        pending()
                        pending = None
                    if norm:
                        runA, runB = runAB[h]
                        if ci == 0:
                            runA = exp_sb[:, ci, :]
                        elif ci < halfA:
                            nxt = pools['attn'].tile(
                                [128, Q], bf16, tag="run", bufs=2,
                                name=f"run_h{h}_c{ci}")
                            nc.vector.tensor_add(nxt[:], runA,
                                                 exp_sb[:, ci, :])
                            runA = nxt[:]
                        elif ci == halfA:
                            runB = exp_sb[:, ci, :]
                        else:
                            nxt = pools['attn'].tile(
                                [128, Q], bf16, tag="run", bufs=2,
                                name=f"run_h{h}_c{ci}")
                            nc.vector.tensor_add(nxt[:], runB,
                                                 exp_sb[:, ci, :])
                            runB = nxt[:]
                        runAB[h] = (runA, runB)
                        if ci == halfA - 1:
                            emit_zmms(h, runA, first=True, last=False)
                    if prev_exp is not None:
                        emit_pv(h - 1, prev_exp, cp_tiles, 2 * ci + 1)
                    return pending

                def make_pending(h):
                    _, runB = runAB[h]
                    def go():
                        emit_zmms(h, runB, first=False, last=True)
                        finish_norm(h)
                    return go

                # ---------------- K projection ----------------
                with tc.tile_pool(name=f"proj{rep}", bufs=1) as proj, \
                     tc.tile_pool(name=f"ppsum{rep}", bufs=2, space="PSUM") as ppsum:
                    mem_sb = proj.tile([128, KC, M], bf16)
                    wk_sb = proj.tile([128, KC, D], bf16)
                    wv_sb = proj.tile([128, KC, D], bf16)
                    qt_sb = proj.tile([128, KC, Q], bf16)
                    wq_sb = proj.tile([128, KC, D], bf16)
                    bvt = proj.tile([128, D], bf16)
                    for c in range(KC):
                        nc.sync.dma_start(out=wk_sb[:, c, :], in_=chunked(wkT[:, :])[:, c, :])
                        nc.sync.dma_start(out=mem_sb[:, c, :], in_=chunked(memT[:, :])[:, c, :])
                    nc.sync.dma_start(out=wv_sb[:], in_=chunked(wvT[:, :]))
                    nc.sync.dma_start(out=qt_sb[:], in_=chunked(qT[:, :]))
                    nc.sync.dma_start(out=wq_sb[:], in_=chunked(wqT[:, :]))
                    # constants needed only later — issue after the streamed
                    # projection inputs so they don't delay the first matmuls
                    nc.scalar.dma_start(out=bvt[:], in_=bv_bc[:, :])

                    # first 4 heads chunk-outer so the PE consumes each
                    # chunk DMA as it lands instead of stalling head 0 on
                    # the full stream; remaining heads run after all data
                    # is resident
                    group = list(range(4))
                    ps_h = {}
                    for h in group:
                        ps_h[h] = ppsum.tile([128, Q], f32, tag="pp",
                                             bufs=4, name=f"kp_h{h}")
                    for c in range(KC):
                        for h in group:
                            hs = slice(h * DH, (h + 1) * DH)
                            for (t0, s0, n) in m_tiles:
                                nc.tensor.matmul(
                                    ps_h[h][:, s0:s0 + n], wk_sb[:, c, hs],
                                    mem_sb[:, c, t0:t0 + n],
                                    start=(c == 0), stop=(c == KC - 1))
                    for h in group:
                        # split drains across ACT and DVE so the next K-proj
                        # heads get their accumulators back ~2x sooner
                        if h % 2 == 0:
                            nc.scalar.activation(
                                k_sb[:, h, :], ps_h[h][:, 0:MK], Identity,
                                bias=bkt[:, h:h + 1])
                        else:
                            with nc.allow_low_precision(reason="bf16 k"):
                                nc.vector.tensor_scalar_add(
                                    k_sb[:, h, :], ps_h[h][:, 0:MK],
                                    bkt[:, h:h + 1])
                    for h in range(4, H):
                        hs = slice(h * DH, (h + 1) * DH)
                        ps = ppsum.tile([128, Q], f32, tag="pp", bufs=4)
                        for c in range(KC):
                            for (t0, s0, n) in m_tiles:
                                nc.tensor.matmul(
                                    ps[:, s0:s0 + n], wk_sb[:, c, hs],
                                    mem_sb[:, c, t0:t0 + n],
                                    start=(c == 0), stop=(c == KC - 1))
                        nc.scalar.activation(
                            k_sb[:, h, :], ps[:, 0:MK], Identity,
                            bias=bkt[:, h:h + 1])

                    # ---- Q projection (all heads), then V projection ----
                    for h in range(H):
                        hs = slice(h * DH, (h + 1) * DH)
                        ps2 = ppsum.tile([128, Q], f32, tag="pp", bufs=4)
                        for c in range(KC):
                            for t in range(NT):
                                ts_ = slice(t * FT, (t + 1) * FT)
                                nc.tensor.matmul(
                                    ps2[:, ts_], wq_sb[:, c, hs],
                                    qt_sb[:, c, ts_],
                                    start=(c == 0), stop=(c == KC - 1))
                        nc.scalar.activation(
                            q_sb[:, h, :], ps2[:], Identity,
                            bias=bqt[:, h:h + 1])
                    for i, mc in enumerate(keep):
                        ms = slice(mc * DH, (mc + 1) * DH)
                        ps = ppsum.tile([128, D], f32, tag="pp", bufs=4)
                        for c in range(KC):
                            for t in range(NT):
                                ts_ = slice(t * FT, (t + 1) * FT)
                                nc.tensor.matmul(
                                    ps[:, ts_], mem_sb[:, c, ms], wv_sb[:, c, ts_],
                                    start=(c == 0), stop=(c == KC - 1))
                        nc.vector.tensor_add(v_sb[:, i, :], ps[:], bvt[:])
                    if wf_sb is not None:
                        # inputs are all streamed by now; wf rides the queues
                        # far ahead of its first use in the final projection
                        nc.scalar.dma_start(out=wf_sb[:],
                                            in_=chunked(wfT[:, :]))

                if 'a' not in parts:
                    continue
                # ---------------- attention (per head) ----------------
                with tc.tile_pool(name=f"attn{rep}", bufs=2) as attn_, \
                     tc.tile_pool(name=f"attn3{rep}", bufs=3) as attn3_, \
                     tc.tile_pool(name=f"dramp{rep}", bufs=2,
                                  space="DRAM") as dramp_, \
                     tc.tile_pool(name=f"spsum{rep}", bufs=2,
                                  space="PSUM") as spsum_, \
                     tc.tile_pool(name=f"cpsum{rep}", bufs=3,
                                  space="PSUM") as cpsum_, \
                     tc.tile_pool(name=f"zpsum{rep}", bufs=1,
                                  space="PSUM") as zpsum_:
                  pools['attn'] = attn_
                  pools['attn3'] = attn3_
                  pools['dramp'] = dramp_
                  pools['st'] = spsum_
                  pools['cp'] = cpsum_
                  pools['zt'] = zpsum_
                  pending = None
                  prev_exp = None
                  p0_queue = list(range(nk)) if norm else []
                  for h in range(H):
                    exp_holder[h] = pools['attn'].tile(
                        [128, nk, Q], bf16,
                        tag="expS0" if h == 0 else "expS",
                        bufs=1 if h == 0 else 3, name=f"exp_h{h}")
                    runAB[h] = (None, None)
                    cp_tiles = [None, None]
                    for ci in range(nk):
                        pending = head_chunk(h, ci, prev_exp, cp_tiles,
                                             pending)
                    if h >= 2 and p0_queue:
                        n_emit = -(-len(p0_queue) // (H - h))  # ceil
                        for _ in range(n_emit):
                            emit_p0_chunk(exp_holder[0], rb_holder[0],
                                          p0_queue.pop(0))
                    pending = make_pending(h)
                    prev_exp = exp_holder[h]

                  # tail: last head's norm chain, PV, then p0 output
                  pending()
                  cp_tiles = [None, None]
                  for idx in range(NT * nk):
                      emit_pv(H - 1, prev_exp, cp_tiles, idx)
                  for ci in p0_queue:
                      emit_p0_chunk(exp_holder[0], rb_holder[0], ci)

            # ---------------- final projection ----------------
            if 'f' not in parts:
                continue
            with tc.tile_pool(name=f"fin{rep}", bufs=3) as fin, \
                 tc.tile_pool(name=f"fpsum{rep}", bufs=3, space="PSUM") as fpsum:
                if wf_sb is None:
                    wf_late = fin.tile([128, KC, D], bf16, bufs=1,
                                       name="wf_late")
                    nc.scalar.dma_start(out=wf_late[:, :, 0:FT],
                                        in_=chunked(wfT[:, :])[:, :, 0:FT])
                    nc.scalar.dma_start(out=wf_late[:, :, FT:D],
                                        in_=chunked(wfT[:, :])[:, :, FT:D])
                    wf_use = wf_late
                else:
                    wf_use = wf_sb
                bft = fin.tile([128, D], bf16, bufs=1)
                nc.sync.dma_start(out=bft[:], in_=bf_bc[:, :])
                # process groups in blocks of 4: all h0..h6 matmuls of a
                # block first, then the four h7 matmuls. The h0..h6 work
                # (~28 matmuls of lead) hides the last head's normalization
                # round-trip, which only ctx[h7] depends on.
                groups = [(qc, t) for qc in range(KC) for t in range(NT)]
                for b0 in range(0, len(groups), 4):
                    blk = groups[b0:b0 + 4]
                    fps = {}
                    for (qc, t) in blk:
                        qs = slice(qc * DH, (qc + 1) * DH)
                        ts_ = slice(t * FT, (t + 1) * FT)
                        fp = fpsum.tile([128, FT], f32, tag="fp", bufs=4,
                                        name=f"fp_{qc}_{t}")
                        fps[(qc, t)] = fp
                        for h in range(H - 1):
                            nc.tensor.matmul(
                                fp[:], ctx_sb[:, h, qs], wf_use[:, h, ts_],
                                start=(h == 0), stop=False)
                    for (qc, t) in blk:
                        qs = slice(qc * DH, (qc + 1) * DH)
                        ts_ = slice(t * FT, (t + 1) * FT)
                        nc.tensor.matmul(
                            fps[(qc, t)][:], ctx_sb[:, H - 1, qs],
                            wf_use[:, H - 1, ts_], start=False, stop=True)
                        of = fin.tile([128, FT], f32, tag="of")
                        nc.vector.tensor_add(of[:], fps[(qc, t)][:],
                                             bft[:, ts_])
                        eng = nc.scalar if (qc + t) % 2 == 0 else nc.sync
                        eng.dma_start(out=wm[qs, ts_], in_=of[:])

    split_sync_waits(nc)
    return nc


def _get_program(reps=1, parts='paf', norm=True, skip=()):
    key = f"nc{reps}_{parts}_{norm}_{sorted(skip)}"
    if key not in _CACHE:
        _CACHE[key] = _build_program(reps, parts, norm, skip)
    return _CACHE[key]


def _host_prep(query, memory, mask, Wk, bk, Wv, bv, Wq, bq, Wf, bf):
    scale = 1.0 / math.sqrt(DH)
    f32 = np.float32

    def t_bf16(a):
        return np.ascontiguousarray(np.asarray(a, dtype=f32).T).astype(_BF16)

    shared = {
        "wkT": t_bf16(Wk),
        "wvT": t_bf16(Wv),
        "wqT": np.ascontiguousarray(
            np.asarray(Wq, dtype=f32).T * f32(scale)).astype(_BF16),
        "wfT": t_bf16(Wf),
        "bk_pp": np.ascontiguousarray(
            np.asarray(bk, dtype=f32).reshape(H, DH).T),
        "bq_pp": np.ascontiguousarray(
            (np.asarray(bq, dtype=f32) * f32(scale)).reshape(H, DH).T),
        "bv_bc": np.ascontiguousarray(
            np.broadcast_to(np.asarray(bv, dtype=f32), (128, D))).astype(_BF16),
        "bf_bc": np.ascontiguousarray(
            np.broadcast_to(np.asarray(bf, dtype=f32), (128, D))).astype(_BF16),
    }
    mask = np.asarray(mask)
    in_maps = []
    for b in range(B):
        mb = np.where(mask[b], f32(-1e30), f32(0.0)).astype(f32)
        in_maps.append({
            **shared,
            "memT": t_bf16(memory[b]),
            "qT": t_bf16(query[b]),
            "mb_pp": np.ascontiguousarray(mb.reshape(KC, DH).T),
        })
    return in_maps


def _skip_set(mask):
    """m-chunks fully masked in EVERY batch element -> provably-dead work."""
    mask = np.asarray(mask).reshape(B, KC, DH)
    full = mask.all(axis=(0, 2))
    skip = tuple(int(c) for c in range(KC) if full[c])
    return skip if len(skip) <= KC - 2 else skip[:KC - 2]


def kernel(query, memory, mask, Wk, bk, Wv, bv, Wq, bq, Wf, bf):
    from concourse.bass_utils import run_bass_kernel_spmd

    nc = _get_program(skip=_skip_set(mask))
    in_maps = _host_prep(query, memory, mask, Wk, bk, Wv, bv, Wq, bq, Wf, bf)
    res = run_bass_kernel_spmd(nc, in_maps, core_ids=list(range(B)))
    wm = np.stack([res.results[b]["wm"] for b in range(B)])
    w0 = np.stack([np.ascontiguousarray(res.results[b]["p0t"].T)
                   for b in range(B)])
    return wm.astype(np.float32), w0.astype(np.float32)
